# revision 25
# baseline (speedup 1.0000x reference)
"""CrossFusionBlock Trainium2 kernel.

Dual-stream cross-attention block (B=8, C=512, HW=1024, 8 heads, FFN 2048).
Sharding: data-parallel over batch across 8 NeuronCores (1 batch element per
core), weights replicated. All weight transposes / permutations / fp8 casts
are done on the host so the device kernel contains no transposes at all.

Cost-model-driven design: matmul cost = out_free_rows x cycles_per_row and
fp8e4+DoubleRow runs at 0.5 cy/row with K=256 per instruction, so every large
matmul (Q/K/V projections, S^T logits, AV, Wo, FFN1, FFN2) uses fp8 DoubleRow.

Layouts (per core, channels-first activations):
  x8      [P, CT, HW] fp8   c = ct*128 + p   (DR contraction pairs = ct pairs)
  q_t/k_t [P, 4, HW]  fp8   head-sliced: slot j=(g,s), partition p:
                            c = 64*(4g + p//32) + 32*s + (p%32)
                            -> S^T DR: lhsT=k_t[32hh:32hh+32, 2g:2g+2, tok128]
  v_hf    [P, TT, H, 72] fp8  token-major V + ones(=8) column for Z
  S^T psum [128tk, 1024tq] -> exp (ACT, scale=1/512; Wq,Wk scaled x8) -> fp8 P^T
  AV fp8 DR -> O/Z via reciprocal broadcast; Wo fp8 DR; LN via PE ones-matmul
  stats + bf16 stt/TSP normalize chains; FFN fp8 DR (W1 x16, undone by gelu
  scale); outputs bf16 (host converts to f32).
"""

import sys

import numpy as np

for _p in ("/opt/trn_rl_repo", "/opt/pypackages"):
    if _p not in sys.path:
        sys.path.insert(0, _p)

import ml_dtypes  # noqa: E402

import concourse.bass as bass  # noqa: E402
from concourse import bacc  # noqa: E402
import concourse.mybir as mybir  # noqa: E402
import concourse.tile as tile  # noqa: E402

P = 128
C = 512
HW = 1024
HEADS = 8
DH = 64
HID = 2048
CT = C // P        # 4 channel tiles
HT = HID // P      # 16 hidden tiles
TT = HW // P       # 8 token tiles
NCH = HW // 512    # 2 free-dim chunks of 512
EPS = 1e-6
BF16 = mybir.dt.bfloat16
FP8 = mybir.dt.float8e4
F32 = mybir.dt.float32
AF = mybir.ActivationFunctionType
ALU = mybir.AluOpType
DR = mybir.MatmulPerfMode.DoubleRow

N_CORES = 8
B, H_IMG, W_IMG = 8, 32, 32

QK_SCALE = 8.0       # host scale on Wq, Wk  (exp scale = 0.125 / QK_SCALE^2)
V_SCALE = 8.0        # host scale on Wv; ones column = V_SCALE cancels it
W1_SCALE = 16.0      # host scale on W1; undone by gelu scale
WO_SCALE = 16.0      # host scale on Wo; undone by the residual stt scalar
                     # (bo is folded into the host-prepped x32)
W2_SCALE = 16.0      # host scale on W2; undone by the residual stt scalar
                     # (b2 is folded into LN1's bias, corrected via b1)
EXP_SCALE = 0.125 / (QK_SCALE * QK_SCALE)

VW = 72  # V row width: DH + ones col + zero pad (16B-aligned for DoubleRow)

# Schraudolph exp-offload: these token-tiles' exp(S^T) runs on DVE as a
# single TensorScalarPtr in the integer domain (bitcast f32 exponent trick),
# read back as stride-2 bf16 (top halves) by the AV matmul. Max rel err
# ~3.3% incl. the bf16 truncation -- same class as the fp8 P^T storage.
OFF_TT = (2, 3)          # whole DoubleRow pair -> AV splits into bf16 singles
SCH_A = (2.0 ** 23 / np.log(2.0)) * EXP_SCALE
SCH_B = float(127 * 2 ** 23 - 336000)
I32 = mybir.dt.int32


# --------------------------------------------------------------------------
# device program
# --------------------------------------------------------------------------

def _emit_proj_one(tc, pools, x8, w, out_t):
    """Channel-major projection: out_t[:, ctj, :] = (w.T @ x)[block ctj].

    All-fp8 DoubleRow: contraction C=512 done in 2 matmuls of K=256.
    """
    nc = tc.nc
    psum_mm = pools["psum_mm"]
    for ctj in range(CT):
        for ch in range(NCH):
            pq = psum_mm.tile([P, 512], F32, tag="mm", name="mm")
            for g2 in range(CT // 2):
                nc.tensor.matmul(
                    pq,
                    lhsT=w[:, 2 * g2:2 * g2 + 2, ctj * P:(ctj + 1) * P],
                    rhs=x8[:, 2 * g2:2 * g2 + 2, ch * 512:(ch + 1) * 512],
                    start=(g2 == 0), stop=(g2 == CT // 2 - 1),
                    perf_mode=DR,
                )
            # ACT is idle during the projection phase; keep DVE free.
            nc.scalar.copy(out=out_t[:, ctj, ch * 512:(ch + 1) * 512], in_=pq)


def _emit_proj_v(tc, pools, x8_kv, wv, v_hf):
    nc = tc.nc
    psum_mm = pools["psum_mm"]
    for tt in range(TT):
        pv = psum_mm.tile([P, 512], F32, tag="mm", name="mm")
        for g2 in range(CT // 2):
            nc.tensor.matmul(
                pv,
                lhsT=x8_kv[:, 2 * g2:2 * g2 + 2, tt * P:(tt + 1) * P],
                rhs=wv[:, 2 * g2:2 * g2 + 2, :],
                start=(g2 == 0), stop=(g2 == CT // 2 - 1),
                perf_mode=DR,
            )
        nc.scalar.copy(
            out=v_hf[:, tt, :, 0:DH],
            in_=pv.rearrange("p (h d) -> p h d", d=DH),
        )
        nc.vector.memset(v_hf[:, tt, :, DH:DH + 1], V_SCALE)


def _emit_st_exp(tc, pools, hp, q_t, k_t, filler=None):
    """S^T for head pair hp via fp8 DoubleRow (dh=64 packed as 32x2), then
    exp(S^T * EXP_SCALE) -> fp8 P^T. Returns per-parity PT tiles."""
    nc = tc.nc
    pt = {}
    for par in (0, 1):
        pt[par] = pools["pt"].tile([P, TT, HW], FP8, tag="pt", name="pt", bufs=3)
        pt[2 + par] = pools["pt"].tile([P, len(OFF_TT), 2 * HW], BF16,
                                       tag="ptb", name="ptb", bufs=3)
    for tt in range(TT):
        if filler is not None:
            filler()
        ps = {}
        for par in (0, 1):
            h = 2 * hp + par
            g, hh = h // 4, h % 4
            p_s = pools["psum_s"].tile([P, HW], F32, tag="s", name="s")
            for ch in range(NCH):
                nc.tensor.matmul(
                    p_s[:, ch * 512:(ch + 1) * 512],
                    lhsT=k_t[32 * hh:32 * hh + 32, 2 * g:2 * g + 2,
                             tt * P:(tt + 1) * P],
                    rhs=q_t[32 * hh:32 * hh + 32, 2 * g:2 * g + 2,
                            ch * 512:(ch + 1) * 512],
                    start=True, stop=True,
                    perf_mode=DR,
                    tile_position=(32 * hh, 0),
                )
            ps[par] = p_s
        for par in (0, 1):
            if tt in OFF_TT:
                j = OFF_TT.index(tt)
                with nc.allow_low_precision(reason="P^T noise class == fp8"):
                    nc.vector.tensor_scalar(
                        out=pt[2 + par][:, j, :].bitcast(I32),
                        in0=ps[par], scalar1=SCH_A, scalar2=SCH_B,
                        op0=ALU.mult, op1=ALU.add,
                    )
            else:
                nc.scalar.activation(out=pt[par][:, tt, :], in_=ps[par],
                                     func=AF.Exp, scale=EXP_SCALE)
    return pt


def _emit_av(tc, pools, hp, pt, v_hf, o_pair, filler=None):
    """AV+Z (ones column = V_SCALE) in fp8 DoubleRow -> normalize into
    o_pair[:, hp] (values are exactly o/z since V and Z carry the same
    scale)."""
    nc = tc.nc
    for par in (0, 1):
        h = 2 * hp + par
        for ch in range(NCH):
            if filler is not None:
                filler()
            sl = slice(ch * 512, (ch + 1) * 512)
            pav = pools["psum_av"].tile([VW, 512], F32, tag="av", name="av")
            mms = []
            for tt2 in range(TT // 2):
                if 2 * tt2 in OFF_TT:
                    for tt in (2 * tt2, 2 * tt2 + 1):
                        j = OFF_TT.index(tt)
                        rhs_bf = pt[2 + par][:, j, :].rearrange(
                            "p (t two) -> p t two", two=2)[:, sl, 1]
                        mms.append(dict(
                            lhsT=v_hf[:, tt, h, :], rhs=rhs_bf, pm=None))
                else:
                    mms.append(dict(
                        lhsT=v_hf[:, 2 * tt2:2 * tt2 + 2, h, :],
                        rhs=pt[par][:, 2 * tt2:2 * tt2 + 2, sl], pm=DR))
            for i, mm in enumerate(mms):
                nc.tensor.matmul(
                    pav, lhsT=mm["lhsT"], rhs=mm["rhs"],
                    start=(i == 0), stop=(i == len(mms) - 1),
                    perf_mode=mm["pm"],
                )
            rz = pools["rz"].tile([P, 512], F32, tag="rz", name="rz", bufs=2)
            nc.vector.reciprocal(out=rz[DH:DH + 1, :], in_=pav[DH:DH + 1, :])
            nc.sync.dma_start(
                out=rz[0:DH, :],
                in_=rz[DH:DH + 1, None, :].to_broadcast((1, DH, 512)),
            )
            if par == 0:
                nc.vector.tensor_tensor(
                    o_pair[0:DH, hp, sl], pav[0:DH, :], rz[0:DH, :], ALU.mult
                )
            else:
                o_tmp = pools["rz"].tile([DH, 512], FP8, tag="o_tmp",
                                         name="o_tmp", bufs=2)
                nc.vector.tensor_tensor(o_tmp, pav[0:DH, :], rz[0:DH, :], ALU.mult)
                nc.sync.dma_start(out=o_pair[DH:P, hp, sl], in_=o_tmp)


def _emit_layernorm(tc, pools, src_bf, w_ap, b_ap, out_writer, inv512, eps_sb,
                    chunks=tuple(range(NCH)), sub_eng=None):
    """LN over the channel (partition x 4-tile) axis of src_bf [P, CT, HW].

    Stats via PE ones-matmuls (bf16), row math on [1,512] rows, bf16
    mu/rs broadcasts, then an all-bf16 stt/TSP normalize chain (4x DVE).
    out_writer(ct, sl, tile_ap, w, b) consumes each normalized [P, 512]
    piece (bf16)."""
    nc = tc.nc
    psum_mm = pools["psum_mm"]
    for ch in chunks:
        sl = slice(ch * 512, (ch + 1) * 512)
        pmu = psum_mm.tile([1, 512], F32, tag="mm", name="mm")
        for k in range(CT):
            nc.tensor.matmul(
                pmu, lhsT=inv512[:, 0:1], rhs=src_bf[:, k, sl],
                start=(k == 0), stop=(k == CT - 1),
            )
        pms = psum_mm.tile([1, 512], F32, tag="mm", name="mm")
        for k in range(CT):
            r2 = pools["sq"].tile([P, 512], BF16, tag="sq", name="sq")
            nc.gpsimd.tensor_tensor(r2, src_bf[:, k, sl], src_bf[:, k, sl], ALU.mult)
            nc.tensor.matmul(
                pms, lhsT=inv512[:, 0:1], rhs=r2,
                start=(k == 0), stop=(k == CT - 1),
            )
        mu_row = pools["rows"].tile([1, 512], BF16, tag="mu_row", name="mu_row", bufs=2)
        rs_row = pools["rows"].tile([1, 512], BF16, tag="rs_row", name="rs_row", bufs=2)
        nc.vector.tensor_copy(out=mu_row, in_=pmu)
        musq = pools["rows"].tile([1, 512], F32, tag="musq", name="musq", bufs=1)
        nc.vector.tensor_tensor(musq, mu_row, mu_row, ALU.mult)
        # var = E[x^2] - mu^2 ; rs = 1/sqrt(var + eps)
        var_row = pools["rows"].tile([1, 512], F32, tag="var_row", name="var_row",
                                     bufs=1)
        nc.vector.tensor_tensor(var_row, pms, musq, ALU.subtract)
        nc.scalar.activation(var_row, var_row, AF.Sqrt, bias=eps_sb[:, 0:1])
        with nc.allow_low_precision(reason="rs broadcast row in bf16 is ample"):
            nc.vector.reciprocal(out=rs_row, in_=var_row)
        mu_b = pools["bcast"].tile([P, 512], BF16, tag="mu_b", name="mu_b", bufs=1)
        rs_b = pools["bcast"].tile([P, 512], BF16, tag="rs_b", name="rs_b", bufs=1)
        nc.sync.dma_start(out=mu_b, in_=mu_row[0:1, None, :].to_broadcast((1, P, 512)))
        nc.sync.dma_start(out=rs_b, in_=rs_row[0:1, None, :].to_broadcast((1, P, 512)))
        for ct in range(CT):
            tmp = pools["tmp"].tile([P, 512], BF16, tag="tmp", name="tmp", bufs=2)
            if sub_eng is not None:
                # Pool engine supports TensorTensor but not TensorScalarPtr.
                sub_eng.tensor_tensor(tmp, src_bf[:, ct, sl], mu_b, ALU.subtract)
            else:
                nc.vector.scalar_tensor_tensor(
                    out=tmp, in0=src_bf[:, ct, sl], scalar=0.0, in1=mu_b,
                    op0=ALU.add, op1=ALU.subtract,
                )
            nc.vector.scalar_tensor_tensor(
                out=tmp, in0=tmp, scalar=0.0, in1=rs_b,
                op0=ALU.add, op1=ALU.mult,
            )
            out_writer(ct, sl, tmp, w_ap(ct), b_ap(ct))


def _emit_wo_residual(tc, pools, pfx, io, cts):
    """Wo projection + bias + residual for the given ct tiles -> r_bf."""
    nc = tc.nc
    o_hf = io["o"]
    x32, wo, params = io["x32"], io["wo"], io["params"]
    psum_mm = pools["psum_mm"]
    if "r" not in io:
        io["r"] = pools["r_pool"].tile([P, CT, HW], BF16, tag=f"r_{pfx}",
                                       name=f"r_{pfx}")
    r_bf = io["r"]
    for ct in cts:
        xr = pools["xr"].tile([P, HW], F32, tag="xr", name="xr")
        nc.sync.dma_start(out=xr, in_=x32[ct * P:(ct + 1) * P, :])
        for ch in range(NCH):
            sl = slice(ch * 512, (ch + 1) * 512)
            pe_ = psum_mm.tile([P, 512], F32, tag="mm", name="mm")
            for i2 in range(HEADS // 4):
                nc.tensor.matmul(
                    pe_,
                    lhsT=wo[:, 2 * i2:2 * i2 + 2, ct * P:(ct + 1) * P],
                    rhs=o_hf[:, 2 * i2:2 * i2 + 2, sl],
                    start=(i2 == 0), stop=(i2 == HEADS // 4 - 1),
                    perf_mode=DR,
                )
            nc.vector.scalar_tensor_tensor(
                out=r_bf[:, ct, sl], in0=pe_, scalar=1.0 / WO_SCALE,
                in1=xr[:, sl], op0=ALU.mult, op1=ALU.add,
            )


def _emit_ln1(tc, pools, pfx, io, chunks=tuple(range(NCH)), sub_eng=None):
    nc = tc.nc
    params = io["params"]
    if "s" not in io:
        io["s"] = pools["s_pool"].tile([P, CT, HW], BF16, tag=f"s_{pfx}",
                                       name=f"s_{pfx}")
        io["s8"] = pools["s_pool"].tile([P, CT, HW], FP8, tag=f"s8_{pfx}",
                                        name=f"s8_{pfx}")
    s_bf = io["s"]
    s_f8 = io["s8"]

    def _ln1_write(ct, sl, tmp, w_scalar, b_scalar):
        nc.vector.tensor_scalar(
            out=s_bf[:, ct, sl], in0=tmp, scalar1=w_scalar, scalar2=b_scalar,
            op0=ALU.mult, op1=ALU.add,
        )
        nc.vector.tensor_copy(out=s_f8[:, ct, sl], in_=s_bf[:, ct, sl])

    _emit_layernorm(
        tc, pools, io["r"],
        lambda ct: params["n1w"][:, ct:ct + 1], lambda ct: params["n1b"][:, ct:ct + 1],
        _ln1_write, io["inv512"], io["eps"], chunks, sub_eng=sub_eng,
    )


def _ffn_chunk_pieces(tc, pools, pfx, io, ch):
    """Thunks emitting the FFN chunk piecewise (16 FFN1-ht + 4 FFN2-ct)."""
    nc = tc.nc
    params = io["params"]
    w1, w2 = io["w1"], io["w2"]
    psum_mm = pools["psum_mm"]
    sl = slice(ch * 512, (ch + 1) * 512)
    state = {}

    def ffn1_piece(ht):
        def f():
            if "h" not in state:
                state["h"] = pools["hbuf"].tile([P, HT, 512], FP8, tag="hbuf",
                                                name="hbuf")
            h_ch = state["h"]
            ph = psum_mm.tile([P, 512], F32, tag="mm", name="mm")
            for g2 in range(CT // 2):
                nc.tensor.matmul(
                    ph,
                    lhsT=w1[:, 2 * g2:2 * g2 + 2, ht * P:(ht + 1) * P],
                    rhs=io["s8"][:, 2 * g2:2 * g2 + 2, sl],
                    start=(g2 == 0), stop=(g2 == CT // 2 - 1),
                    perf_mode=DR,
                )
            nc.scalar.activation(
                out=h_ch[:, ht, :], in_=ph, func=AF.Gelu,
                bias=params["b1"][:, ht:ht + 1], scale=1.0 / W1_SCALE,
            )
        return f

    def ffn2_piece(ct):
        def f():
            if "r2" not in io:
                io["r2"] = pools["r_pool"].tile([P, CT, HW], BF16, tag=f"r_{pfx}",
                                                name=f"r2_{pfx}")
            r2_bf = io["r2"]
            h_ch = state["h"]
            pf = psum_mm.tile([P, 512], F32, tag="mm", name="mm")
            for g2 in range(HT // 2):
                nc.tensor.matmul(
                    pf,
                    lhsT=w2[:, 2 * g2:2 * g2 + 2, ct * P:(ct + 1) * P],
                    rhs=h_ch[:, 2 * g2:2 * g2 + 2, :],
                    start=(g2 == 0), stop=(g2 == HT // 2 - 1),
                    perf_mode=DR,
                )
            nc.vector.scalar_tensor_tensor(
                out=r2_bf[:, ct, sl], in0=pf, scalar=1.0 / W2_SCALE,
                in1=io["s"][:, ct, sl], op0=ALU.mult, op1=ALU.add,
            )
        return f

    return [ffn1_piece(ht) for ht in range(HT)] + [ffn2_piece(ct) for ct in range(CT)]


def _emit_ffn_chunk(tc, pools, pfx, io, ch):
    for piece in _ffn_chunk_pieces(tc, pools, pfx, io, ch):
        piece()


def _emit_ln2(tc, pools, pfx, io, chunks=tuple(range(NCH)), sub_eng=None):
    nc = tc.nc
    params, out_dram = io["params"], io["out"]

    def _ln2_write(ct, sl, tmp, w_scalar, b_scalar):
        o_bf = pools["ostage"].tile([P, 512], BF16, tag="ostage", name="ostage",
                                    bufs=2)
        nc.vector.tensor_scalar(
            out=o_bf, in0=tmp, scalar1=w_scalar, scalar2=b_scalar,
            op0=ALU.mult, op1=ALU.add,
        )
        nc.sync.dma_start(out=out_dram[ct * P:(ct + 1) * P, sl], in_=o_bf)

    _emit_layernorm(
        tc, pools, io["r2"],
        lambda ct: params["n2w"][:, ct:ct + 1], lambda ct: params["n2b"][:, ct:ct + 1],
        _ln2_write, io["inv512"], io["eps"], chunks, sub_eng=sub_eng,
    )


def build_program():
    nc = bacc.Bacc("TRN2", target_bir_lowering=False, debug=False)

    def din(name, shape, dt):
        return nc.dram_tensor(name, list(shape), dt, kind="ExternalInput").ap()

    x32 = {p: din(f"x_{p}32", (C, HW), F32) for p in "sf"}
    x8d = {p: din(f"x_{p}8", (C, HW), FP8) for p in "sf"}
    wq8 = {p: din(f"{p}_wq8", (C, C), FP8) for p in "sf"}
    wk8 = {p: din(f"{p}_wk8", (C, C), FP8) for p in "sf"}
    wv8 = {p: din(f"{p}_wv8", (C, C), FP8) for p in "sf"}
    wo8 = {p: din(f"{p}_wo8", (C, C), FP8) for p in "sf"}
    w18 = {p: din(f"{p}_w18", (C, HID), FP8) for p in "sf"}
    w28 = {p: din(f"{p}_w28", (HID, C), FP8) for p in "sf"}
    pnames = ("n1w", "n1b", "n2w", "n2b")
    prm = {
        p: {n: din(f"{p}_{n}", (P, CT), F32) for n in pnames} for p in "sf"
    }
    for p in "sf":
        prm[p]["b1"] = din(f"{p}_b1", (P, HT), F32)
    outs = {
        p: nc.dram_tensor(f"out_{p}", [C, HW], BF16, kind="ExternalOutput").ap()
        for p in "sf"
    }

    with tile.TileContext(nc) as tc:
        from contextlib import ExitStack
        with ExitStack() as ctx:
            pools = {}

            def pool(name, bufs, space="SBUF", stack=None):
                pools[name] = (stack or ctx).enter_context(
                    tc.tile_pool(name=name, bufs=bufs, space=space)
                )
                return pools[name]

            # whole-program pools
            pool("psum_mm", 2, space="PSUM")
            pool("psum_s", 2, space="PSUM")
            pool("psum_av", 2, space="PSUM")
            pool("consts", 1)
            pool("params", 1)
            pool("xr", 1)
            pool("rows", 1)
            pool("bcast", 1)
            pool("tmp", 1)
            pool("sq", 2)
            pool("rz", 1)
            pool("pt", 34)
            pool("r_pool", 1)
            pool("s_pool", 1)
            pool("hbuf", 1)
            pool("ostage", 2)
            pool("wffn", 1)

            inv512 = pools["consts"].tile([P, 1], BF16)
            nc.vector.memset(inv512, 1.0 / C)
            eps_sb = pools["consts"].tile([1, 1], F32)
            nc.vector.memset(eps_sb, EPS)

            # ---- load params (small) ----
            params = {}
            for p in "sf":
                params[p] = {}
                for n, ap_ in prm[p].items():
                    t = pools["params"].tile(list(ap_.shape), F32, tag=f"{p}_{n}")
                    nc.sync.dma_start(out=t, in_=ap_)
                    params[p][n] = t

            # ---- pools with manual lifetimes (LIFO discipline) ----
            owo_stack = ctx.enter_context(ExitStack())
            pool("o_pool", 1, stack=owo_stack)
            pool("wo_pool", 1, stack=owo_stack)
            qkv_stack = ctx.enter_context(ExitStack())
            pool("qkv", 1, stack=qkv_stack)
            xw_stack = ctx.enter_context(ExitStack())
            pool("x8", 1, stack=xw_stack)
            pool("wproj", 1, stack=xw_stack)

            def load_wproj(p, nm, srcw, eng):
                t = pools["wproj"].tile([P, CT, C], FP8, tag=nm, name=f"{nm}_{p}")
                eng.dma_start(
                    out=t, in_=srcw.rearrange("(ct p) o -> p ct o", p=P)
                )
                return t

            def load_x8(p, eng):
                t = pools["x8"].tile([P, CT, HW], FP8, tag=f"x8_{p}",
                                     name=f"x8_{p}")
                eng.dma_start(
                    out=t, in_=x8d[p].rearrange("(ct p) t -> p ct t", p=P)
                )
                return t

            # Q(s) needs only x_s + wq_s: emit those DMAs first so the first
            # projection matmuls start early in the input stream.
            x8 = {"s": load_x8("s", nc.sync)}
            wq_s = load_wproj("s", "wq", wq8["s"], nc.gpsimd)
            x8["f"] = load_x8("f", nc.gpsimd)

            qkv = {}
            for p in "sf":
                qkv[f"q_{p}"] = pools["qkv"].tile(
                    [P, CT, HW], FP8, tag=f"q_{p}", name=f"q_{p}")
                qkv[f"k_{p}"] = pools["qkv"].tile(
                    [P, CT, HW], FP8, tag=f"k_{p}", name=f"k_{p}")
                qkv[f"v_{p}"] = pools["qkv"].tile(
                    [P, TT, HEADS, VW], FP8, tag=f"v_{p}", name=f"v_{p}")
                nc.vector.memset(qkv[f"v_{p}"][:, :, :, DH + 1:], 0.0)

            wo_sb = {}
            o_sb = {}
            for p in "sf":
                wo_sb[p] = pools["wo_pool"].tile([P, CT, C], FP8, tag=f"wo_{p}",
                                                 name=f"wo_{p}")
                o_sb[p] = pools["o_pool"].tile([P, HEADS // 2, HW], FP8,
                                               tag=f"o_{p}", name=f"o_{p}")

            def load_wo(p):
                nc.sync.dma_start(
                    out=wo_sb[p],
                    in_=wo8[p].rearrange("(ct p) o -> p ct o", p=P),
                )

            ios = {}
            for p in "sf":
                ios[p] = {
                    "o": o_sb[p], "x32": x32[p], "wo": wo_sb[p],
                    "params": params[p], "out": outs[p],
                    "inv512": inv512, "eps": eps_sb,
                }

            # software-pipelined attention: S^T+exp of pair N overlaps
            # AV of pair N-1 on PE, so PE never waits on the ACT exp chain.
            # stream 's': q from x_s, kv from x_f ; stream 'f': swapped
            seq = [("s", hp) for hp in range(4)] + [("f", hp) for hp in range(4)]
            pts = {}

            def st(i):
                p, hp = seq[i]
                pts[i] = _emit_st_exp(tc, pools, hp, qkv[f"q_{p}"], qkv[f"k_{p}"])

            def av(i):
                p, hp = seq[i]
                _emit_av(tc, pools, hp, pts.pop(i), qkv[f"v_{p}"], o_sb[p])

            # ---- A(s) ----
            _emit_proj_one(tc, pools, x8["s"], wq_s, qkv["q_s"])
            _emit_proj_one(tc, pools, x8["f"],
                           load_wproj("s", "wk", wk8["s"], nc.gpsimd),
                           qkv["k_s"])
            _emit_proj_v(tc, pools, x8["f"],
                         load_wproj("s", "wv", wv8["s"], nc.gpsimd),
                         qkv["v_s"])

            # ---- B(s) | A(f) ----
            st(0)
            _emit_proj_one(tc, pools, x8["f"],
                           load_wproj("f", "wq", wq8["f"], nc.gpsimd),
                           qkv["q_f"])
            _emit_proj_one(tc, pools, x8["s"],
                           load_wproj("f", "wk", wk8["f"], nc.gpsimd),
                           qkv["k_f"])
            st(1)
            av(0)
            _emit_proj_v(tc, pools, x8["s"],
                         load_wproj("f", "wv", wv8["f"], nc.gpsimd),
                         qkv["v_f"])
            load_wo("s")
            st(2)
            av(1)
            load_wo("f")
            st(3)
            av(2)
            xw_stack.close()

            def load_wffn(p):
                t1 = pools["wffn"].tile([P, CT, HID], FP8, tag="w1", name="w1")
                nc.gpsimd.dma_start(
                    out=t1, in_=w18[p].rearrange("(ct p) o -> p ct o", p=P)
                )
                t2 = pools["wffn"].tile([P, HT, C], FP8, tag="w2", name="w2")
                nc.gpsimd.dma_start(
                    out=t2, in_=w28[p].rearrange("(ht p) o -> p ht o", p=P)
                )
                return t1, t2

            ios["s"]["w1"], ios["s"]["w2"] = load_wffn("s")

            # ---- B(f) | C(s) | D(s) ----
            st(4)
            av(3)
            _emit_wo_residual(tc, pools, "s", ios["s"], (0, 1))
            st(5)
            av(4)
            _emit_wo_residual(tc, pools, "s", ios["s"], (2, 3))
            st(6)
            av(5)
            _emit_ln1(tc, pools, "s", ios["s"], chunks=(0,))
            st(7)
            av(6)
            _emit_ln1(tc, pools, "s", ios["s"], chunks=(1,))
            _emit_ffn_chunk(tc, pools, "s", ios["s"], 0)
            av(7)
            _emit_ffn_chunk(tc, pools, "s", ios["s"], 1)
            qkv_stack.close()

            # ---- C(f) | LN2(s); then D(f) ----
            _emit_wo_residual(tc, pools, "f", ios["f"], (0, 1))
            _emit_wo_residual(tc, pools, "f", ios["f"], (2, 3))
            _emit_ln1(tc, pools, "f", ios["f"], chunks=(0,))
            _emit_ln2(tc, pools, "s", ios["s"], chunks=(0,), sub_eng=nc.gpsimd)
            _emit_ln1(tc, pools, "f", ios["f"], chunks=(1,))
            ios["f"]["w1"], ios["f"]["w2"] = load_wffn("f")
            _emit_ln2(tc, pools, "s", ios["s"], chunks=(1,), sub_eng=nc.gpsimd)
            _emit_ffn_chunk(tc, pools, "f", ios["f"], 0)
            _emit_ln2(tc, pools, "f", ios["f"], chunks=(0,), sub_eng=nc.gpsimd)
            _emit_ffn_chunk(tc, pools, "f", ios["f"], 1)
            _emit_ln2(tc, pools, "f", ios["f"], chunks=(1,), sub_eng=nc.gpsimd)

    nc.compile()
    return nc


# --------------------------------------------------------------------------
# host side
# --------------------------------------------------------------------------

_BF = ml_dtypes.bfloat16
_F8 = ml_dtypes.float8_e4m3


def _qk_out_perm():
    """Column permutation for Wq/Wk so the projection psum lands in the
    S^T-DoubleRow layout: column j = (g*2+s)*128 + p holds channel
    c = 64*(4g + p//32) + 32*s + (p%32)."""
    j = np.arange(C)
    g = j // 256
    s = (j // 128) % 2
    p = j % 128
    return 64 * (4 * g + p // 32) + 32 * s + (p % 32)


_QK_PERM = _qk_out_perm()


def _prep_shared_inputs(inputs):
    """Host-side weight prep: transposes, permutations, fp8 casts."""
    sh = {}
    for p, ap in (("s", "s_"), ("f", "f_")):
        wq, wk, wv, wo = (inputs[ap + n] for n in ("Wq", "Wk", "Wv", "Wo"))
        sh[f"{p}_wq8"] = np.ascontiguousarray(
            (QK_SCALE * wq.T)[:, _QK_PERM]).astype(_F8)
        sh[f"{p}_wk8"] = np.ascontiguousarray(
            (QK_SCALE * wk.T)[:, _QK_PERM]).astype(_F8)
        sh[f"{p}_wv8"] = np.ascontiguousarray(V_SCALE * wv.T).astype(_F8)
        sh[f"{p}_wo8"] = np.ascontiguousarray(WO_SCALE * wo.T).astype(_F8)
        w1 = inputs[f"{p}ffn_W1"]
        w2 = inputs[f"{p}ffn_W2"]
        b2 = inputs[f"{p}ffn_b2"].astype(np.float64)
        sh[f"{p}_w18"] = np.ascontiguousarray(W1_SCALE * w1.T).astype(_F8)
        sh[f"{p}_w28"] = np.ascontiguousarray(W2_SCALE * w2.T).astype(_F8)
        n1w, n1b = (f"{p}n1_w", f"{p}n1_b")
        n2w, n2b = (f"{p}n2_w", f"{p}n2_b")
        sh[f"{p}_n1w"] = np.ascontiguousarray(inputs[n1w].reshape(CT, P).T).astype(np.float32)
        # b2 is folded into LN1's bias (s' = s + b2); b1 gets the matching
        # -W1 @ b2 correction so FFN1 still sees the un-shifted s.
        n1b_eff = np.asarray(inputs[n1b], np.float64) + b2
        sh[f"{p}_n1b"] = np.ascontiguousarray(
            n1b_eff.reshape(CT, P).T).astype(np.float32)
        sh[f"{p}_n2w"] = np.ascontiguousarray(inputs[n2w].reshape(CT, P).T).astype(np.float32)
        sh[f"{p}_n2b"] = np.ascontiguousarray(inputs[n2b].reshape(CT, P).T).astype(np.float32)
        b1_eff = (np.asarray(inputs[f"{p}ffn_b1"], np.float64)
                  - np.asarray(w1, np.float64) @ b2)
        sh[f"{p}_b1"] = np.ascontiguousarray(
            b1_eff.reshape(HT, P).T
        ).astype(np.float32)
    return sh


def _rename_ln(inputs):
    """Map reference param names (sn1_w...) onto the scheme used above."""
    out = dict(inputs)
    for p in "sf":
        for i in "12":
            for wb in "wb":
                out[f"{p}n{i}_{wb}"] = inputs[f"{p}n{i}_{wb}"]
    return out


def make_in_maps(inputs):
    inputs = _rename_ln(inputs)
    shared = _prep_shared_inputs(inputs)
    xs = np.ascontiguousarray(inputs["spatial_feat"].reshape(B, C, HW))
    xf = np.ascontiguousarray(inputs["freq_feat"].reshape(B, C, HW))
    in_maps = []
    # bo is folded into the residual source: r = (x + bo) + enh
    bo_s = np.asarray(inputs["s_bo"], np.float32)[:, None]
    bo_f = np.asarray(inputs["f_bo"], np.float32)[:, None]
    for b in range(N_CORES):
        m = dict(shared)
        m["x_s32"] = np.ascontiguousarray(xs[b] + bo_s).astype(np.float32)
        m["x_f32"] = np.ascontiguousarray(xf[b] + bo_f).astype(np.float32)
        m["x_s8"] = xs[b].astype(_F8)
        m["x_f8"] = xf[b].astype(_F8)
        in_maps.append(m)
    return in_maps


_CACHED = {}


def _get_program():
    if "nc" not in _CACHED:
        _CACHED["nc"] = build_program()
    return _CACHED["nc"]


def run_on_hw(inputs, trace=False, trace_kwargs=None):
    from concourse.bass_utils import run_bass_kernel_spmd

    nc = _get_program()
    in_maps = make_in_maps(inputs)
    res = run_bass_kernel_spmd(
        nc, in_maps, list(range(N_CORES)), trace=trace,
        **(dict(trace_kwargs=trace_kwargs) if trace_kwargs else {}),
    )
    s = np.stack([np.asarray(res.results[b]["out_s"]).astype(np.float32)
                  for b in range(B)])
    f = np.stack([np.asarray(res.results[b]["out_f"]).astype(np.float32)
                  for b in range(B)])
    s = s.reshape(B, C, H_IMG, W_IMG)
    f = f.reshape(B, C, H_IMG, W_IMG)
    return (s, f), res


def kernel(**inputs):
    out, _ = run_on_hw(inputs, trace=False)
    return out
